# revision 1
# baseline (speedup 1.0000x reference)
"""Trainium2 Bass kernel for the octonion causal self-attention block.

Strategy (8 NeuronCores, SPMD):
  Each core owns one octonion component c (= heads 2c, 2c+1).
  - Host prep: ternary-quantize weights exactly as the reference does,
    assemble the effective [C, C] block matrices (sign/permutation
    combine folded in), permute q/k output channels into RoPE
    split-layout (real|imag halves), fold 1/sqrt(HD) into wq, and
    transpose x to channel-major xT.  All heavy FLOPs stay on device.
  - Device phase 1: projections qT/kT (channel-major) + v (natural) for
    the core's component, streaming xT once; RoPE applied on the fly.
  - Device phase 2: causal attention for the core's 4 (batch, head)
    pairs: S^T = K^T Q per s-tile, exp (no max subtraction -- scores are
    O(1) bounded), triangular mask on the diagonal tile, P^T V with an
    appended ones-column in V giving the softmax denominator for free,
    per-partition normalize, PE-transpose back to channel-major yT
    (kept in SBUF).
  - Device phase 3: PARTIAL output projection: out_partial^T =
    Wo[rows of component c]^T @ yT_c -- needs only local data, so no
    collective at all.  The host sums the 8 partial [C, B*T] outputs
    while unsharding (that sum is the tensor-parallel "unshard").
"""

import numpy as np
import ml_dtypes

import concourse.bass as bass
import concourse.tile as tile
from concourse import bacc, mybir
from concourse.bass_utils import run_bass_kernel_spmd
from concourse.masks import make_identity

# ---------------------------------------------------------------- problem dims
B, T_FULL, C, H = 2, 2048, 2048, 16
HD = C // H          # 128
P = C // 8           # 256
N_CORES = 8
KT = C // 128        # 16 contraction k-tiles

OCT_SIGN = np.array([
    [1, 1, 1, 1, 1, 1, 1, 1],
    [1,-1, 1,-1, 1,-1,-1, 1],
    [1,-1,-1, 1, 1, 1,-1,-1],
    [1, 1,-1,-1, 1,-1, 1,-1],
    [1,-1,-1,-1,-1, 1, 1, 1],
    [1, 1,-1, 1,-1,-1,-1, 1],
    [1, 1, 1,-1,-1, 1,-1,-1],
    [1,-1, 1, 1,-1,-1, 1,-1]], dtype=np.float32)
OCT_IDX = np.array([
    [0,1,2,3,4,5,6,7],
    [1,0,3,2,5,4,7,6],
    [2,3,0,1,6,7,4,5],
    [3,2,1,0,7,6,5,4],
    [4,5,6,7,0,1,2,3],
    [5,4,7,6,1,0,3,2],
    [6,7,4,5,2,3,0,1],
    [7,6,5,4,3,2,1,0]], dtype=np.int32)
_COMB = np.zeros((8, 8, 8), np.float32)
for _i in range(8):
    for _j in range(8):
        _COMB[OCT_IDX[_i, _j], _i, _j] = OCT_SIGN[_i, _j]

BF16 = ml_dtypes.bfloat16


# ------------------------------------------------------------------- host prep
def _ternary(W: np.ndarray) -> np.ndarray:
    """Exact replica of the reference's ternary quantization (fwd value)."""
    try:
        import jax
        import jax.numpy as jnp
        cpu = jax.local_devices(backend="cpu")[0]
        with jax.default_device(cpu):
            Wj = jnp.asarray(W)
            s = jnp.mean(jnp.abs(Wj)) + 1e-8
            q = jnp.round(jnp.clip(Wj / s, -1.0, 1.0)) * s
            return np.asarray(q)
    except Exception:
        s = np.float32(np.mean(np.abs(W.astype(np.float32)))) + np.float32(1e-8)
        return (np.rint(np.clip(W / s, -1.0, 1.0)) * s).astype(np.float32)


def _build_w_eff(W: np.ndarray) -> np.ndarray:
    """[8,P,P] component weights -> effective [C, C] with octonion combine."""
    Wq = _ternary(W)  # (8, P, P)
    # W_eff[(j,p),(k,q)] = sum_i COMB[k,i,j] * Wq[i,p,q]; exactly one i per (j,k)
    return np.einsum("kij,ipq->jpkq", _COMB, Wq).reshape(C, C).astype(np.float32)


def _rope_colperm() -> np.ndarray:
    """colperm[new] = old: within each head, [re0..re63 | im0..im63]."""
    perm = np.zeros(C, dtype=np.int64)
    for h in range(H):
        base = h * HD
        for r in range(HD // 2):
            perm[base + r] = base + 2 * r
            perm[base + HD // 2 + r] = base + 2 * r + 1
    return perm


def prep_inputs(inputs: dict, T: int) -> list[dict]:
    """Build the 8 per-core input maps from the full problem inputs."""
    NT = B * T
    x = np.asarray(inputs["x"], np.float32)[:, :T, :]
    cos = np.asarray(inputs["freqs_cos"], np.float32)[:T]   # [T, 64]
    sin = np.asarray(inputs["freqs_sin"], np.float32)[:T]

    wq_eff = _build_w_eff(np.asarray(inputs["wq"], np.float32))
    wk_eff = _build_w_eff(np.asarray(inputs["wk"], np.float32))
    wv_eff = _build_w_eff(np.asarray(inputs["wv"], np.float32))
    wo_eff = _build_w_eff(np.asarray(inputs["wo"], np.float32))

    perm = _rope_colperm()
    wq_eff = wq_eff[:, perm] * np.float32(HD ** -0.5)
    wk_eff = wk_eff[:, perm]

    # xT [C, NT] -> k-tiles [KT, 128, NT]
    xt = np.ascontiguousarray(
        x.reshape(NT, C).T.reshape(KT, 128, NT).astype(BF16))

    # rope tables, duplicated-half layout [128, T]
    cosd = np.empty((128, T), np.float32)
    cosd[0:64] = cos.T
    cosd[64:128] = cos.T
    sind = np.empty((128, T), np.float32)
    sind[0:64] = -sin.T
    sind[64:128] = sin.T
    cosd = cosd.astype(BF16)
    sind = sind.astype(BF16)

    tri = np.triu(np.ones((128, 128), np.float32)).astype(BF16)  # [s,q] s<=q

    def blocks(w_eff: np.ndarray, c: int) -> np.ndarray:
        blk = w_eff[:, c * P:(c + 1) * P]                  # [C, 256]
        return np.ascontiguousarray(blk.reshape(KT, 128, P).astype(BF16))

    in_maps = []
    for c in range(N_CORES):
        # o-proj row-block for component c: [256, C] -> [2, 128, C]
        wo_rows = np.ascontiguousarray(
            wo_eff[c * P:(c + 1) * P, :].reshape(2, 128, C).astype(BF16))
        in_maps.append({
            "xt": xt,
            "wq": blocks(wq_eff, c),
            "wk": blocks(wk_eff, c),
            "wv": blocks(wv_eff, c),
            "wo": wo_rows,
            "cosd": cosd,
            "sind": sind,
            "tri": tri,
        })
    return in_maps


# ------------------------------------------------------------- device program
def build_nc(T: int = T_FULL, n_cores: int = N_CORES):
    NT = B * T
    ST = T // 128            # s-tiles per batch
    NST = NT // 128
    TCH = min(512, T)        # token chunk; must not cross a batch boundary
    NCH = NT // TCH
    bf16 = mybir.dt.bfloat16
    f32 = mybir.dt.float32

    nc = bacc.Bacc("TRN2", target_bir_lowering=False, debug=False,
                   num_devices=n_cores)

    xt_d = nc.dram_tensor("xt", [KT, 128, NT], bf16, kind="ExternalInput")
    wq_d = nc.dram_tensor("wq", [KT, 128, P], bf16, kind="ExternalInput")
    wk_d = nc.dram_tensor("wk", [KT, 128, P], bf16, kind="ExternalInput")
    wv_d = nc.dram_tensor("wv", [KT, 128, P], bf16, kind="ExternalInput")
    wo_d = nc.dram_tensor("wo", [2, 128, C], bf16, kind="ExternalInput")
    cos_d = nc.dram_tensor("cosd", [128, T], bf16, kind="ExternalInput")
    sin_d = nc.dram_tensor("sind", [128, T], bf16, kind="ExternalInput")
    tri_d = nc.dram_tensor("tri", [128, 128], bf16, kind="ExternalInput")
    out_d = nc.dram_tensor("outt", [C, NT], bf16, kind="ExternalOutput")

    with tile.TileContext(nc) as tc:
        with (
            tc.tile_pool(name="consts", bufs=1) as consts,
            tc.tile_pool(name="persist", bufs=1) as persist,
        ):
            # ================= phase 1: projections + rope =================
            with (
                nc.named_scope("proj"),
                tc.tile_pool(name="xts", bufs=2) as xts_pool,
                tc.tile_pool(name="rope", bufs=3) as rope_pool,
                tc.tile_pool(name="ps1", bufs=3, space="PSUM") as ps1,
                tc.tile_pool(name="psv", bufs=3, space="PSUM") as psv,
            ):
                # ---- resident constants
                wq_s = consts.tile([128, KT, P], bf16, tag="wq")
                wk_s = consts.tile([128, KT, P], bf16, tag="wk")
                wv_s = consts.tile([128, KT, P], bf16, tag="wv")
                wo_s = consts.tile([128, 2, C], bf16, tag="wo")
                # DMA order tuned to the consumption schedule: wq+xt0 in
                # interleaved k-quarters (first matmuls start after ~1/4),
                # then cos/sin (rope frees the proj PSUM slots), wk, the
                # chunk-1 xt prefetch, and only then the later-used weights.
                xt0_s = xts_pool.tile([128, KT, TCH], bf16, tag="xt")
                for kq in range(0, KT, 4):
                    nc.sync.dma_start(
                        out=wq_s[:, kq:kq + 4, :],
                        in_=wq_d.ap()[kq:kq + 4].rearrange("k p n -> p k n"))
                    nc.sync.dma_start(
                        out=xt0_s[:, kq:kq + 4, :],
                        in_=xt_d.ap()[kq:kq + 4, :, 0:TCH]
                        .rearrange("k p n -> p k n"))
                cos_s = consts.tile([128, T], bf16, tag="cos")
                sin_s = consts.tile([128, T], bf16, tag="sin")
                nc.sync.dma_start(out=cos_s, in_=cos_d.ap())
                nc.sync.dma_start(out=sin_s, in_=sin_d.ap())
                nc.sync.dma_start(out=wk_s,
                                  in_=wk_d.ap().rearrange("k p n -> p k n"))
                xt1_s = None
                if NCH > 1:
                    xt1_s = xts_pool.tile([128, KT, TCH], bf16, tag="xt")
                    nc.sync.dma_start(out=xt1_s,
                                      in_=xt_d.ap()[:, :, TCH:2 * TCH]
                                      .rearrange("k p n -> p k n"))
                nc.sync.dma_start(out=wv_s,
                                  in_=wv_d.ap().rearrange("k p n -> p k n"))
                tri_s = consts.tile([128, 128], bf16, tag="tri")
                nc.sync.dma_start(out=tri_s, in_=tri_d.ap())
                nc.sync.dma_start(out=wo_s,
                                  in_=wo_d.ap().rearrange("k p n -> p k n"))
                ident = consts.tile([128, 128], bf16, tag="ident")
                make_identity(nc, ident[:])

                # ---- persistent activations
                qt_s = persist.tile([128, 2, NT], bf16, tag="qt")  # [d, head, tok]
                kt_s = persist.tile([128, 2, NT], bf16, tag="kt")
                v_s = persist.tile([128, NST, 2, 132], bf16, tag="v")
                nc.vector.memset(v_s[:, :, :, 128:129], 1.0)

                for ch in range(NCH):
                    t0 = ch * TCH
                    pos0 = t0 % T          # position within batch
                    if ch == 0:
                        xt_s = xt0_s
                    elif ch == 1:
                        xt_s = xt1_s
                    else:
                        xt_s = xts_pool.tile([128, KT, TCH], bf16, tag="xt")
                        nc.sync.dma_start(out=xt_s,
                                          in_=xt_d.ap()[:, :, t0:t0 + TCH]
                                          .rearrange("k p n -> p k n"))

                    # q/k projections (channel-major out) + rope
                    for w_s, dst in ((wq_s, qt_s), (wk_s, kt_s)):
                        for a in range(2):  # head within component
                            ps_q = ps1.tile([128, TCH], f32, tag="psq")
                            for k in range(KT):
                                nc.tensor.matmul(
                                    ps_q[:],
                                    lhsT=w_s[:, k, a * 128:(a + 1) * 128],
                                    rhs=xt_s[:, k, :],
                                    start=(k == 0), stop=(k == KT - 1))
                            # rope: out = q * cos_dup + swap(q) * sin_signed
                            # (swap of partition halves must go through DMA --
                            # compute engines cannot move data across partitions)
                            q_sb = rope_pool.tile([128, TCH], bf16, tag="qsb")
                            nc.scalar.copy(out=q_sb[:], in_=ps_q[:])
                            qsw = rope_pool.tile([128, TCH], bf16, tag="qsw")
                            nc.sync.dma_start(out=qsw[0:64, :], in_=q_sb[64:128, :])
                            nc.sync.dma_start(out=qsw[64:128, :], in_=q_sb[0:64, :])
                            t1 = rope_pool.tile([128, TCH], bf16, tag="t1")
                            nc.vector.tensor_mul(
                                t1[:], q_sb[:], cos_s[:, pos0:pos0 + TCH])
                            t2 = rope_pool.tile([128, TCH], bf16, tag="t2")
                            nc.vector.tensor_mul(
                                t2[:], qsw[:], sin_s[:, pos0:pos0 + TCH])
                            nc.vector.tensor_add(
                                dst[:, a, t0:t0 + TCH], t1[:], t2[:])

                    # v projection (natural layout)
                    for st in range(TCH // 128):
                        stg = t0 // 128 + st
                        ps_v = psv.tile([128, P], f32, tag="psv")
                        for k in range(KT):
                            nc.tensor.matmul(
                                ps_v[:],
                                lhsT=xt_s[:, k, st * 128:(st + 1) * 128],
                                rhs=wv_s[:, k, :],
                                start=(k == 0), stop=(k == KT - 1))
                        # [t, (head d)] -> v_s[:, stg, head, 0:128]
                        nc.vector.tensor_copy(
                            v_s[:, stg, :, 0:128],
                            ps_v[:].rearrange("p (a d) -> p a d", a=2))

            # ====== phases 2+3: causal attention + partial o-proj, per batch
            # o-proj for batch b is emitted right after batch b's attention,
            # so its matmuls fill attention-phase gaps and its 32MB output
            # DMA spreads over the rest of the kernel instead of the tail.
            ystages = {}
            with (
                tc.tile_pool(name="pt", bufs=ST) as pt_pool,
                tc.tile_pool(name="att_small", bufs=4) as small_pool,
                tc.tile_pool(name="ysb", bufs=ST) as ysb_pool,
                tc.tile_pool(name="ostage", bufs=4) as o_pool,
                tc.tile_pool(name="ps_s", bufs=3, space="PSUM") as ps_s,
                tc.tile_pool(name="ps_y", bufs=3, space="PSUM") as ps_y,
                tc.tile_pool(name="ps_t", bufs=2, space="PSUM") as ps_t,
            ):
                for b in range(B):
                    for a in range(2):
                        qh = qt_s[:, a, b * T:(b + 1) * T]   # [128, T]
                        kh = kt_s[:, a, b * T:(b + 1) * T]
                        # --- A: scores^T + exp per s-tile
                        pts = [None] * ST
                        for j in range(ST):
                            pt_j = pt_pool.tile([128, T], bf16, tag="ptj")
                            pts[j] = pt_j
                            q0 = 128 * j
                            while q0 < T:
                                w = min(512, T - q0)
                                ps = ps_s.tile([128, 512], f32, tag="pss")
                                nc.tensor.matmul(
                                    ps[:, 0:w],
                                    lhsT=kh[:, 128 * j:128 * (j + 1)],
                                    rhs=qh[:, q0:q0 + w],
                                    start=True, stop=True)
                                nc.scalar.activation(
                                    out=pt_j[:, q0:q0 + w], in_=ps[:, 0:w],
                                    func=mybir.ActivationFunctionType.Exp)
                                q0 += w
                            # causal mask on the diagonal 128x128 block
                            nc.vector.tensor_mul(
                                pt_j[:, 128 * j:128 * (j + 1)],
                                pt_j[:, 128 * j:128 * (j + 1)], tri_s[:])
                        # --- B: y = P^T.T @ [v|1], normalize, transpose
                        y_stage = persist.tile([128, T], bf16,
                                               tag=f"ystage{b}{a}")
                        ystages[(b, a)] = y_stage
                        y_sbs = [None] * ST
                        # descending i: long accumulation chains first, so the
                        # PE stays ahead of the DVE normalize chain throughout
                        for i in reversed(range(ST)):
                            psy = ps_y.tile([128, 132], f32, tag="psy")
                            for j in range(i + 1):
                                nc.tensor.matmul(
                                    psy[:, 0:129],
                                    lhsT=pts[j][:, 128 * i:128 * (i + 1)],
                                    rhs=v_s[:, b * ST + j, a, 0:129],
                                    start=(j == 0), stop=(j == i))
                            recip = small_pool.tile([128, 1], f32, tag="recip")
                            nc.vector.reciprocal(recip[:], psy[:, 128:129])
                            y_sb = ysb_pool.tile([128, 128], bf16, tag="ysb")
                            nc.vector.tensor_scalar_mul(
                                y_sb[:], psy[:, 0:128], recip[:])
                            y_sbs[i] = y_sb
                        # transposes batched after the PV chain so the PE
                        # never stalls waiting on the DVE normalize
                        for i in range(ST):
                            pst = ps_t.tile([128, 128], bf16, tag="pst")
                            nc.tensor.transpose(pst[:], y_sbs[i][:], ident[:])
                            # NB: must stay on DVE -- ACT reading bf16 PSUM
                            # hard-faulted the exec unit on HW
                            nc.vector.tensor_copy(
                                y_stage[:, 128 * i:128 * (i + 1)], pst[:])

                    # --- partial o-proj for this batch:
                    # outT_partial[cout, t] = sum_{cin in c} Wo[cin,cout] yT[cin,t]
                    for lch in range(T // TCH):
                        lt0 = lch * TCH
                        t0 = b * T + lt0
                        for m in range(C // 128):        # 16 cout tiles
                            ps = ps_s.tile([128, 512], f32, tag="pss")
                            for k in range(2):           # cin k-tiles (= heads)
                                nc.tensor.matmul(
                                    ps[:, 0:TCH],
                                    lhsT=wo_s[:, k, m * 128:(m + 1) * 128],
                                    rhs=ystages[(b, k)][:, lt0:lt0 + TCH],
                                    start=(k == 0), stop=(k == 1))
                            # copies split DVE/ACT: ACT's next exp stream is
                            # sequenced behind these matmuls anyway, so it is
                            # idle here, and DVE alone (~6.2us/chunk) sits
                            # right at the PE rate (~6.8us/chunk)
                            o_sb = o_pool.tile([128, TCH], bf16, tag="osb")
                            if m % 2 == 0:
                                nc.vector.tensor_copy(o_sb[:], ps[:, 0:TCH])
                            else:
                                nc.scalar.copy(out=o_sb[:], in_=ps[:, 0:TCH])
                            nc.sync.dma_start(
                                out=out_d.ap()[m * 128:(m + 1) * 128,
                                               t0:t0 + TCH],
                                in_=o_sb[:])

    nc.compile()
    return nc


# ------------------------------------------------------------------ entrypoint
_NC_CACHE: dict = {}


def _get_nc(T: int):
    if T not in _NC_CACHE:
        _NC_CACHE[T] = build_nc(T)
    return _NC_CACHE[T]


def assemble_output(results: list[dict], T: int = T_FULL) -> np.ndarray:
    # unshard = sum of the 8 tensor-parallel partial projections (bf16 -> f32)
    outT = results[0]["outt"].astype(np.float32)                # [C, NT]
    for r in results[1:]:
        outT += r["outt"].astype(np.float32)
    return np.ascontiguousarray(outT.T).reshape(B, T, C).astype(np.float32)


def kernel(**inputs) -> np.ndarray:
    nc = _get_nc(T_FULL)
    in_maps = prep_inputs(inputs, T_FULL)
    res = run_bass_kernel_spmd(nc, in_maps, list(range(N_CORES)))
    return assemble_output(res.results, T_FULL)



# revision 12
# speedup vs baseline: 1.0865x; 1.0865x over previous
"""Trainium2 Bass kernel for the octonion causal self-attention block.

Strategy (8 NeuronCores, SPMD, tensor-parallel over octonion components):
  Each core owns one octonion component c (= heads 2c, 2c+1) and computes
  q/k/v projections, causal attention for its 4 (batch, head) pairs, and a
  PARTIAL output projection; the host sums the 8 partial [C, NT] outputs.

Key design points (v2, software-pipelined):
  - All heavy tensors host-packed partition-major so every DMA is a clean
    2D descriptor ([128, KT, *] weights / x, [128, 2, C] wo).
  - PV runs V-stationary: lhsT = V_j [s,d], rhs streams pt_j[:, q] into
    PSUM y-slabs [d, 1024] -> y lands channel-major (no PE transposes, no
    per-tile DVE copies), and pt tiles can be freed after one sweep.
  - Softmax denominator: DVE accumulates ptsum = sum_j pt_j in bf16; one
    ones[128,128] matmul per 512-chunk partition-sums AND broadcasts it to
    PSUM; DVE reciprocal -> recipb; DVE multiply normalizes y during the
    PSUM->SBUF move.
  - Global software pipeline keeps the PE stream gapless (TRN2 PE drops
    from 2.4 GHz to 1.2 GHz after any stall; full clock needs ~3us of
    continuous execution): each stretch has ONE safe-ordered core stream
    (PV(p, slab1, j) strictly before S(p+1, j) so pt-slot reuse never
    creates a cross-engine wait cycle) plus filler streams (proj chunks
    6-7, o-proj m-tiles) woven in by a fair weaver that preserves
    per-stream order.
  - o-proj output staged per 512-token chunk in one [128, 16, 512] SBUF
    tile -> a single DMA per chunk (16 dispatches total instead of 128).
"""

import numpy as np
import ml_dtypes

import concourse.bass as bass
import concourse.tile as tile
from concourse import bacc, mybir
from concourse.bass_utils import run_bass_kernel_spmd

# ---------------------------------------------------------------- problem dims
B, T_FULL, C, H = 2, 2048, 2048, 16
HD = C // H          # 128
P = C // 8           # 256
N_CORES = 8
KT = C // 128        # 16 contraction k-tiles

OCT_SIGN = np.array([
    [1, 1, 1, 1, 1, 1, 1, 1],
    [1,-1, 1,-1, 1,-1,-1, 1],
    [1,-1,-1, 1, 1, 1,-1,-1],
    [1, 1,-1,-1, 1,-1, 1,-1],
    [1,-1,-1,-1,-1, 1, 1, 1],
    [1, 1,-1, 1,-1,-1,-1, 1],
    [1, 1, 1,-1,-1, 1,-1,-1],
    [1,-1, 1, 1,-1,-1, 1,-1]], dtype=np.float32)
OCT_IDX = np.array([
    [0,1,2,3,4,5,6,7],
    [1,0,3,2,5,4,7,6],
    [2,3,0,1,6,7,4,5],
    [3,2,1,0,7,6,5,4],
    [4,5,6,7,0,1,2,3],
    [5,4,7,6,1,0,3,2],
    [6,7,4,5,2,3,0,1],
    [7,6,5,4,3,2,1,0]], dtype=np.int32)
_COMB = np.zeros((8, 8, 8), np.float32)
for _i in range(8):
    for _j in range(8):
        _COMB[OCT_IDX[_i, _j], _i, _j] = OCT_SIGN[_i, _j]

BF16 = ml_dtypes.bfloat16


# ------------------------------------------------------------------- host prep
def _ternary(W: np.ndarray) -> np.ndarray:
    """Exact replica of the reference's ternary quantization (fwd value)."""
    try:
        import jax
        import jax.numpy as jnp
        cpu = jax.local_devices(backend="cpu")[0]
        with jax.default_device(cpu):
            Wj = jnp.asarray(W)
            s = jnp.mean(jnp.abs(Wj)) + 1e-8
            q = jnp.round(jnp.clip(Wj / s, -1.0, 1.0)) * s
            return np.asarray(q)
    except Exception:
        s = np.float32(np.mean(np.abs(W.astype(np.float32)))) + np.float32(1e-8)
        return (np.rint(np.clip(W / s, -1.0, 1.0)) * s).astype(np.float32)


def _build_w_eff(W: np.ndarray) -> np.ndarray:
    """[8,P,P] component weights -> effective [C, C] with octonion combine."""
    Wq = _ternary(W)  # (8, P, P)
    return np.einsum("kij,ipq->jpkq", _COMB, Wq).reshape(C, C).astype(np.float32)


def _rope_colperm() -> np.ndarray:
    """colperm[new] = old: within each head, [re0..re63 | im0..im63]."""
    perm = np.zeros(C, dtype=np.int64)
    for h in range(H):
        base = h * HD
        for r in range(HD // 2):
            perm[base + r] = base + 2 * r
            perm[base + HD // 2 + r] = base + 2 * r + 1
    return perm


def prep_inputs(inputs: dict, T: int) -> list[dict]:
    """Build the 8 per-core input maps from the full problem inputs."""
    NT = B * T
    x = np.asarray(inputs["x"], np.float32)[:, :T, :]
    cos = np.asarray(inputs["freqs_cos"], np.float32)[:T]   # [T, 64]
    sin = np.asarray(inputs["freqs_sin"], np.float32)[:T]

    wq_eff = _build_w_eff(np.asarray(inputs["wq"], np.float32))
    wk_eff = _build_w_eff(np.asarray(inputs["wk"], np.float32))
    wv_eff = _build_w_eff(np.asarray(inputs["wv"], np.float32))
    wo_eff = _build_w_eff(np.asarray(inputs["wo"], np.float32))

    perm = _rope_colperm()
    wq_eff = wq_eff[:, perm] * np.float32(HD ** -0.5)
    wk_eff = wk_eff[:, perm]

    # xT [C, NT] packed partition-major -> [128, KT, NT]
    xt = np.ascontiguousarray(
        x.reshape(NT, C).T.reshape(KT, 128, NT).transpose(1, 0, 2).astype(BF16))

    # rope tables, duplicated-half layout [128, T]
    cosd = np.empty((128, T), np.float32)
    cosd[0:64] = cos.T
    cosd[64:128] = cos.T
    sind = np.empty((128, T), np.float32)
    sind[0:64] = -sin.T
    sind[64:128] = sin.T
    cosd = cosd.astype(BF16)
    sind = sind.astype(BF16)

    tri = np.triu(np.ones((128, 128), np.float32)).astype(BF16)  # [s,q] s<=q

    def blocks(w_eff: np.ndarray, c: int) -> np.ndarray:
        blk = w_eff[:, c * P:(c + 1) * P]                  # [C, 256]
        return np.ascontiguousarray(
            blk.reshape(KT, 128, P).transpose(1, 0, 2).astype(BF16))

    in_maps = []
    for c in range(N_CORES):
        # o-proj row-block for component c: [256, C] -> [128, 2, C]
        wo_rows = np.ascontiguousarray(
            wo_eff[c * P:(c + 1) * P, :].reshape(2, 128, C)
            .transpose(1, 0, 2).astype(BF16))
        in_maps.append({
            "xt": xt,
            "wq": blocks(wq_eff, c),
            "wk": blocks(wk_eff, c),
            "wv": blocks(wv_eff, c),
            "wo": wo_rows,
            "cosd": cosd,
            "sind": sind,
            "tri": tri,
        })
    return in_maps


# ------------------------------------------------------------------- weaver
def weave(*streams):
    """Fairly interleave unit streams by cumulative weight.

    Each stream is a list of (weight, fn). Executes every fn exactly once,
    preserving per-stream order, picking at each step the least-progressed
    stream by weight fraction.
    """
    streams = [s for s in streams if s]
    total = [max(sum(w for w, _ in s), 1e-9) for s in streams]
    done = [0.0] * len(streams)
    idx = [0] * len(streams)
    while True:
        cand = [i for i in range(len(streams)) if idx[i] < len(streams[i])]
        if not cand:
            break
        i = min(cand, key=lambda i: done[i] / total[i])
        w, fn = streams[i][idx[i]]
        idx[i] += 1
        done[i] += w
        fn()


# ------------------------------------------------------------- device program
def build_nc(T: int = T_FULL, n_cores: int = N_CORES):
    NT = B * T
    ST = T // 128            # 16 s-tiles per batch
    NST = NT // 128          # 32
    TCH = 512                # token chunk
    NCH = NT // TCH          # 8
    SLAB = 1024              # y-accumulator slab width (2 psum banks)
    bf16 = mybir.dt.bfloat16
    f32 = mybir.dt.float32
    EXP = mybir.ActivationFunctionType.Exp

    nc = bacc.Bacc("TRN2", target_bir_lowering=False, debug=False,
                   num_devices=n_cores)

    xt_d = nc.dram_tensor("xt", [128, KT, NT], bf16, kind="ExternalInput")
    wq_d = nc.dram_tensor("wq", [128, KT, P], bf16, kind="ExternalInput")
    wk_d = nc.dram_tensor("wk", [128, KT, P], bf16, kind="ExternalInput")
    wv_d = nc.dram_tensor("wv", [128, KT, P], bf16, kind="ExternalInput")
    wo_d = nc.dram_tensor("wo", [128, 2, C], bf16, kind="ExternalInput")
    cos_d = nc.dram_tensor("cosd", [128, T], bf16, kind="ExternalInput")
    sin_d = nc.dram_tensor("sind", [128, T], bf16, kind="ExternalInput")
    tri_d = nc.dram_tensor("tri", [128, 128], bf16, kind="ExternalInput")
    out_d = nc.dram_tensor("outt", [C, NT], bf16, kind="ExternalOutput")

    with tile.TileContext(nc) as tc:
        with (
            tc.tile_pool(name="consts", bufs=1) as consts,
            tc.tile_pool(name="persist", bufs=1) as persist,
            tc.tile_pool(name="ptlo", bufs=2) as ptlo,       # pt j=0..3
            tc.tile_pool(name="pthi", bufs=1) as pthi,       # pt j=4..15
            tc.tile_pool(name="ptsum", bufs=2) as ptsum_pool,
            tc.tile_pool(name="recipb", bufs=1) as recipb_pool,
            tc.tile_pool(name="sps", bufs=2, space="PSUM") as sps,
            tc.tile_pool(name="yslab", bufs=1, space="PSUM") as yslab_pool,
            tc.tile_pool(name="denb", bufs=1, space="PSUM") as denb_pool,
        ):
            # ---------------- persistent SBUF state
            wo_s = consts.tile([128, 2, C], bf16, tag="wo")
            tri_s = consts.tile([128, 128], bf16, tag="tri")
            ones_s = consts.tile([128, 128], bf16, tag="ones")
            qt_s = persist.tile([128, 2, NT], bf16, tag="qt")  # [d, head, tok]
            kt_s = persist.tile([128, 2, NT], bf16, tag="kt")
            v_s = persist.tile([128, NST, 2, 128], bf16, tag="v")
            yst = [persist.tile([128, T], bf16, tag=f"yst{p}",
                                name=f"yst{p}") for p in range(4)]

            # mutable emission state
            st = {"pt": {}, "ptsum": {}, "recipb": {}, "yslab": {},
                  "xt": {}, "cs": {}, "osb": {}}

            # ============ emission helpers =================================
            def S_chunks(p, j, clo, chi, with_ptsum):
                """Emit S^T matmul+exp for chunks clo..chi of tile j, plus the
                diag mask when its chunk is covered and (optionally) the
                ptsum accumulate once the tile is complete."""
                b, a = p // 2, p % 2
                base = b * T
                kh = kt_s[:, a, base + 128 * j: base + 128 * (j + 1)]
                pt = st["pt"][(p, j)]
                for cc in range(max(clo, j // 4), chi + 1):
                    q0 = max(512 * cc, 128 * j)
                    q1 = 512 * (cc + 1)
                    w = q1 - q0
                    ps = sps.tile([128, 512], f32, tag="ps_s")
                    nc.tensor.matmul(
                        ps[:, 0:w], lhsT=kh,
                        rhs=qt_s[:, a, base + q0: base + q1],
                        start=True, stop=True)
                    nc.scalar.activation(
                        out=pt[:, q0 - 128 * j: q1 - 128 * j],
                        in_=ps[:, 0:w], func=EXP)
                    if cc == j // 4:
                        # causal mask on the diagonal 128 block
                        nc.vector.tensor_mul(pt[:, 0:128], pt[:, 0:128],
                                             tri_s[:])
                # ptsum accumulate (bf16; validated vs reference)
                if with_ptsum:
                    if j == 0:
                        pts = ptsum_pool.tile([128, T], bf16, tag="ptsum")
                        st["ptsum"][p] = pts
                        nc.vector.tensor_copy(pts[:, :], pt[:, :])
                    else:
                        pts = st["ptsum"][p]
                        nc.vector.tensor_add(pts[:, 128 * j:T],
                                             pts[:, 128 * j:T], pt[:, :])

            def S_unit(p, j, sink, cmax=3):
                """S tile j: pt alloc + chunks up to cmax (use cmax<3 when the
                last q-chunk's qt/kt writers are woven into the same stretch,
                then emit the rest via S_tail after those writers)."""
                def u(p=p, j=j, cmax=cmax):
                    pt_pool = ptlo if j < 4 else pthi
                    ptw = T - 128 * j
                    pt = pt_pool.tile([128, ptw], bf16, tag=f"pt{j}")
                    st["pt"][(p, j)] = pt
                    S_chunks(p, j, 0, cmax, with_ptsum=(cmax == 3))
                sink.append((0.85 if j < 4 else (0.64 if j < 8 else 0.32), u))

            def S_tail(p, cdone):
                """Sequentially finish chunks cdone+1..3 for every tile."""
                for j in range(16):
                    S_chunks(p, j, cdone + 1, 3, with_ptsum=True)

            def PV_unit(p, slab, j, sink):
                """V-stationary PV matmuls for tile j into the y-slab."""
                def u(p=p, slab=slab, j=j):
                    b, a = p // 2, p % 2
                    qlo = slab * SLAB
                    jmax = 8 if slab == 0 else 16
                    if j == 0:
                        ys = yslab_pool.tile([128, SLAB], f32, tag="yslab")
                        st["yslab"][(p, slab)] = ys
                    ys = st["yslab"][(p, slab)]
                    pt = st["pt"][(p, j)]
                    for cc in range(max(2 * slab, j // 4), 2 * slab + 2):
                        q0 = max(512 * cc, 128 * j)
                        q1 = 512 * (cc + 1)
                        nc.tensor.matmul(
                            ys[:, q0 - qlo: q1 - qlo],
                            lhsT=v_s[:, b * ST + j, a, :],
                            rhs=pt[:, q0 - 128 * j: q1 - 128 * j],
                            start=(j == 0),
                            stop=(j == min(jmax - 1, 4 * cc + 3)))
                w = (512 * (2 * slab + 2) - max(512 * 2 * slab, 128 * j)) \
                    / 2400.0
                sink.append((max(w, 0.05), u))

            def denb_unit(p, slab, sink):
                """ones-matmul partition-sum+broadcast of ptsum, then recip."""
                def u(p=p, slab=slab):
                    rb = recipb_pool.tile([128, SLAB], f32, tag="recipb")
                    st["recipb"][(p, slab)] = rb
                    pts = st["ptsum"][p]
                    for cc in range(2 * slab, 2 * slab + 2):
                        dn = denb_pool.tile([128, 512], f32, tag="denb")
                        nc.tensor.matmul(dn[:], lhsT=ones_s[:],
                                         rhs=pts[:, 512 * cc: 512 * (cc + 1)],
                                         start=True, stop=True)
                        nc.vector.reciprocal(
                            rb[:, 512 * cc - slab * SLAB:
                               512 * (cc + 1) - slab * SLAB], dn[:])
                sink.append((1.0, u))

            def ymul_unit(p, slab, sink):
                """Normalize the finished y-slab into the ystage (DVE)."""
                def u(p=p, slab=slab):
                    ys = st["yslab"][(p, slab)]
                    rb = st["recipb"][(p, slab)]
                    nc.vector.tensor_mul(
                        yst[p][:, slab * SLAB:(slab + 1) * SLAB], ys[:], rb[:])
                # weight inflated so the weaver inserts filler after it,
                # giving the DVE time before the next slab's first matmul
                sink.append((2.0, u))

            def attn_core(p, next_S, next_cmax=3):
                """Safe-ordered core stream for one stretch.

                PV(p, slab1, j) strictly precedes S(p+1, j) for j>=4 so the
                single-buffered pt slot reuse never creates a wait cycle.
                """
                s = []
                if next_S is not None:
                    S_unit(next_S, 0, s, next_cmax)
                    S_unit(next_S, 1, s, next_cmax)
                for j in range(8):
                    PV_unit(p, 0, j, s)
                if next_S is not None:
                    S_unit(next_S, 2, s, next_cmax)
                    S_unit(next_S, 3, s, next_cmax)
                denb_unit(p, 0, s)
                ymul_unit(p, 0, s)
                for j in range(4):
                    PV_unit(p, 1, j, s)
                for j in range(4, 16):
                    PV_unit(p, 1, j, s)
                    if next_S is not None:
                        S_unit(next_S, j, s, next_cmax)
                denb_unit(p, 1, s)
                ymul_unit(p, 1, s)
                return s

            # ================= phase 1: projections =========================
            with (
                tc.tile_pool(name="projw", bufs=1) as projw,
                tc.tile_pool(name="xts", bufs=2) as xts_pool,
                tc.tile_pool(name="cstile", bufs=2) as cs_pool,
                tc.tile_pool(name="rope", bufs=2) as rope_pool,
                tc.tile_pool(name="ps1", bufs=2, space="PSUM") as ps1,
                tc.tile_pool(name="psv", bufs=1, space="PSUM") as psv,
            ):
                wq_s = projw.tile([128, KT, P], bf16, tag="wq")
                wk_s = projw.tile([128, KT, P], bf16, tag="wk")
                wv_s = projw.tile([128, KT, P], bf16, tag="wv")

                def load_xt(ch):
                    xs = xts_pool.tile([128, KT, TCH], bf16, tag="xt")
                    st["xt"][ch] = xs
                    nc.sync.dma_start(
                        out=xs, in_=xt_d.ap()[:, :, ch * TCH:(ch + 1) * TCH])

                def load_cs(ch):
                    pos0 = (ch * TCH) % T
                    cch = cs_pool.tile([128, TCH], bf16, tag="cos")
                    sch = cs_pool.tile([128, TCH], bf16, tag="sin")
                    st["cs"][ch] = (cch, sch)
                    nc.sync.dma_start(out=cch,
                                      in_=cos_d.ap()[:, pos0:pos0 + TCH])
                    nc.sync.dma_start(out=sch,
                                      in_=sin_d.ap()[:, pos0:pos0 + TCH])

                # --- preload DMAs, consumption-ordered
                xt0 = xts_pool.tile([128, KT, TCH], bf16, tag="xt")
                st["xt"][0] = xt0
                for kq in range(0, KT, 4):
                    nc.sync.dma_start(out=wq_s[:, kq:kq + 4, :],
                                      in_=wq_d.ap()[:, kq:kq + 4, :])
                    nc.sync.dma_start(out=xt0[:, kq:kq + 4, :],
                                      in_=xt_d.ap()[:, kq:kq + 4, 0:TCH])
                load_cs(0)
                nc.sync.dma_start(out=wk_s, in_=wk_d.ap())
                load_xt(1)
                nc.sync.dma_start(out=wv_s, in_=wv_d.ap())
                nc.sync.dma_start(out=tri_s, in_=tri_d.ap())
                nc.vector.memset(ones_s[:], 1.0)
                nc.sync.dma_start(out=wo_s, in_=wo_d.ap())

                def proj_units(ch):
                    """8 units interleaved qk/v: each v chain is followed by
                    a q/k chain so the single-buffered psv copy hides."""
                    t0 = ch * TCH
                    qk_units = []
                    for a in range(2):
                        for w_s, dst in ((wq_s, qt_s), (wk_s, kt_s)):
                            def u(w_s=w_s, dst=dst, a=a, t0=t0, ch=ch):
                                xt_sb = st["xt"][ch]
                                cch, sch = st["cs"][ch]
                                ps = ps1.tile([128, TCH], f32, tag="psq")
                                for k in range(KT):
                                    nc.tensor.matmul(
                                        ps[:],
                                        lhsT=w_s[:, k, a * 128:(a + 1) * 128],
                                        rhs=xt_sb[:, k, :],
                                        start=(k == 0), stop=(k == KT - 1))
                                q_sb = rope_pool.tile([128, TCH], bf16,
                                                      tag="qsb")
                                nc.scalar.copy(out=q_sb[:], in_=ps[:])
                                # partition-half swap must go through DMA
                                qsw = rope_pool.tile([128, TCH], bf16,
                                                     tag="qsw")
                                nc.sync.dma_start(out=qsw[0:64, :],
                                                  in_=q_sb[64:128, :])
                                nc.sync.dma_start(out=qsw[64:128, :],
                                                  in_=q_sb[0:64, :])
                                d = dst[:, a, t0:t0 + TCH]
                                nc.vector.tensor_mul(d, q_sb[:], cch[:])
                                t2 = rope_pool.tile([128, TCH], bf16, tag="t2")
                                nc.vector.tensor_mul(t2[:], qsw[:], sch[:])
                                nc.vector.tensor_add(d, d, t2[:])
                            qk_units.append((3.6, u))
                    v_units = []
                    for stt in range(TCH // 128):
                        def u(stt=stt, t0=t0, ch=ch):
                            xt_sb = st["xt"][ch]
                            stg = t0 // 128 + stt
                            ps = psv.tile([128, P], f32, tag="psv")
                            for k in range(KT):
                                nc.tensor.matmul(
                                    ps[:],
                                    lhsT=xt_sb[:, k, stt * 128:(stt + 1) * 128],
                                    rhs=wv_s[:, k, :],
                                    start=(k == 0), stop=(k == KT - 1))
                            nc.vector.tensor_copy(
                                v_s[:, stg, :, :],
                                ps[:].rearrange("p (a d) -> p a d", a=2))
                        v_units.append((1.8, u))
                    units = []
                    for i in range(4):
                        units.append(qk_units[i])
                        units.append(v_units[i])
                    return units

                def S_stream(p, jlo, jhi):
                    s = []
                    for j in range(jlo, jhi):
                        S_unit(p, j, s)
                    return s

                # --- chunks 0..3 (batch 0) straight through
                # NB: the xt prefetch for ch+2 must be EMITTED after chunk
                # ch's units -- the pool slot-rotation dependency only covers
                # accesses emitted so far, so emitting the DMA first lets it
                # overwrite xt[ch] while chunk ch's matmuls still read it.
                for ch in range(4):
                    if ch + 1 < NCH:
                        load_cs(ch + 1)
                    for _, u in proj_units(ch):
                        u()
                    if ch + 2 < NCH:
                        load_xt(ch + 2)

                # --- chunks 4,5 woven with S(0)
                load_cs(5)
                weave(proj_units(4), S_stream(0, 0, 8))
                load_xt(6)
                load_cs(6)
                weave(proj_units(5), S_stream(0, 8, 16))
                load_xt(7)

                # --- stretch A: PV(0) + S(1) + proj ch6
                load_cs(7)
                weave(attn_core(0, 1), proj_units(6))

                # --- stretch B: PV(1) + S(2) + proj ch7.
                # S(2)'s last q-chunk reads qt/kt tokens written by ch7's
                # rope (same stretch), so those chunks are deferred to a
                # sequential tail emitted after the whole weave.
                weave(attn_core(1, 2, next_cmax=2), proj_units(7))
                S_tail(2, 2)

            # ============ phases 2+3: attention tail + o-proj ===============
            with (
                tc.tile_pool(name="osb", bufs=2) as osb_pool,
                tc.tile_pool(name="opj", bufs=3, space="PSUM") as opj,
            ):
                def oproj_units(b, lchs, dve_frac):
                    """o-proj m-tile chains for token chunks lchs of batch b."""
                    units = []
                    for lch in lchs:
                        for m in range(16):
                            def u(b=b, lch=lch, m=m, dve_frac=dve_frac):
                                if m == 0:
                                    osb = osb_pool.tile([128, 16, 512], bf16,
                                                        tag="osb")
                                    st["osb"][(b, lch)] = osb
                                osb = st["osb"][(b, lch)]
                                po = opj.tile([128, 512], f32, tag="po")
                                q0 = lch * 512
                                for a in range(2):
                                    nc.tensor.matmul(
                                        po[:],
                                        lhsT=wo_s[:, a, m * 128:(m + 1) * 128],
                                        rhs=yst[2 * b + a][:, q0:q0 + 512],
                                        start=(a == 0), stop=(a == 1))
                                if (m % 4) < 4 * dve_frac:
                                    nc.vector.tensor_copy(osb[:, m, :], po[:])
                                else:
                                    nc.scalar.copy(out=osb[:, m, :], in_=po[:])
                                if m == 15:
                                    tc0 = b * T + lch * 512
                                    nc.sync.dma_start(
                                        out=out_d.ap()[:, tc0:tc0 + 512]
                                        .rearrange("(m p) t -> p m t", p=128),
                                        in_=osb[:])
                            units.append((0.43, u))
                    return units

                # --- stretch C: PV(2) + S(3) + o-proj(b0) chunks 0,1
                weave(attn_core(2, 3), oproj_units(0, (0, 1), 0.75))

                # --- stretch D: PV(3) slab0 + o-proj(b0) chunks 2,3
                sD = []
                for j in range(8):
                    PV_unit(3, 0, j, sD)
                denb_unit(3, 0, sD)
                ymul_unit(3, 0, sD)
                weave(sD, oproj_units(0, (2, 3), 0.5))

                # --- stretch E: PV(3) slab1 + o-proj(b1) chunks 0,1
                # (chunks 2,3 need ymul(3,1), the last core unit, so they
                # must come after the whole core stream -- weaving them in
                # would let the PE block on a DVE op whose own PE deps sit
                # behind the blocked instruction)
                sE = []
                for j in range(16):
                    PV_unit(3, 1, j, sE)
                denb_unit(3, 1, sE)
                ymul_unit(3, 1, sE)
                weave(sE, oproj_units(1, (0, 1), 0.5))
                for _, u in oproj_units(1, (2, 3), 0.5):
                    u()

    nc.compile()
    return nc


# ------------------------------------------------------------------ entrypoint
_NC_CACHE: dict = {}


def _get_nc(T: int):
    if T not in _NC_CACHE:
        _NC_CACHE[T] = build_nc(T)
    return _NC_CACHE[T]


def assemble_output(results: list[dict], T: int = T_FULL) -> np.ndarray:
    # unshard = sum of the 8 tensor-parallel partial projections (bf16 -> f32)
    outT = results[0]["outt"].astype(np.float32)                # [C, NT]
    for r in results[1:]:
        outT += r["outt"].astype(np.float32)
    return np.ascontiguousarray(outT.T).reshape(B, T, C).astype(np.float32)


def kernel(**inputs) -> np.ndarray:
    nc = _get_nc(T_FULL)
    in_maps = prep_inputs(inputs, T_FULL)
    res = run_bass_kernel_spmd(nc, in_maps, list(range(N_CORES)))
    return assemble_output(res.results, T_FULL)


# revision 17
# speedup vs baseline: 1.2606x; 1.1602x over previous
"""Trainium2 Bass kernel for the octonion causal self-attention block.

Strategy (8 NeuronCores, SPMD, tensor-parallel over octonion components):
  Each core owns one octonion component c (= heads 2c, 2c+1) and computes
  q/k/v projections, causal attention for its 4 (batch, head) pairs, and a
  PARTIAL output projection; the host sums the 8 partial [C, NT] outputs.

Key design points (v2, software-pipelined):
  - All heavy tensors host-packed partition-major so every DMA is a clean
    2D descriptor ([128, KT, *] weights / x, [128, 2, C] wo).
  - PV runs V-stationary: lhsT = V_j [s,d], rhs streams pt_j[:, q] into
    PSUM y-slabs [d, 1024] -> y lands channel-major (no PE transposes, no
    per-tile DVE copies), and pt tiles can be freed after one sweep.
  - Softmax denominator: DVE accumulates ptsum = sum_j pt_j in bf16; one
    ones[128,128] matmul per 512-chunk partition-sums AND broadcasts it to
    PSUM; DVE reciprocal -> recipb; DVE multiply normalizes y during the
    PSUM->SBUF move.
  - Global software pipeline keeps the PE stream gapless (TRN2 PE drops
    from 2.4 GHz to 1.2 GHz after any stall; full clock needs ~3us of
    continuous execution): each stretch has ONE safe-ordered core stream
    (PV(p, slab1, j) strictly before S(p+1, j) so pt-slot reuse never
    creates a cross-engine wait cycle) plus filler streams (proj chunks
    6-7, o-proj m-tiles) woven in by a fair weaver that preserves
    per-stream order.
  - o-proj output staged per 512-token chunk in one [128, 16, 512] SBUF
    tile -> a single DMA per chunk (16 dispatches total instead of 128).
"""

import numpy as np
import ml_dtypes

import concourse.bass as bass
import concourse.tile as tile
from concourse import bacc, mybir
from concourse.bass_utils import run_bass_kernel_spmd

# ---------------------------------------------------------------- problem dims
B, T_FULL, C, H = 2, 2048, 2048, 16
HD = C // H          # 128
P = C // 8           # 256
N_CORES = 8
KT = C // 128        # 16 contraction k-tiles

OCT_SIGN = np.array([
    [1, 1, 1, 1, 1, 1, 1, 1],
    [1,-1, 1,-1, 1,-1,-1, 1],
    [1,-1,-1, 1, 1, 1,-1,-1],
    [1, 1,-1,-1, 1,-1, 1,-1],
    [1,-1,-1,-1,-1, 1, 1, 1],
    [1, 1,-1, 1,-1,-1,-1, 1],
    [1, 1, 1,-1,-1, 1,-1,-1],
    [1,-1, 1, 1,-1,-1, 1,-1]], dtype=np.float32)
OCT_IDX = np.array([
    [0,1,2,3,4,5,6,7],
    [1,0,3,2,5,4,7,6],
    [2,3,0,1,6,7,4,5],
    [3,2,1,0,7,6,5,4],
    [4,5,6,7,0,1,2,3],
    [5,4,7,6,1,0,3,2],
    [6,7,4,5,2,3,0,1],
    [7,6,5,4,3,2,1,0]], dtype=np.int32)
_COMB = np.zeros((8, 8, 8), np.float32)
for _i in range(8):
    for _j in range(8):
        _COMB[OCT_IDX[_i, _j], _i, _j] = OCT_SIGN[_i, _j]

BF16 = ml_dtypes.bfloat16


# ------------------------------------------------------------------- host prep
def _ternary(W: np.ndarray) -> np.ndarray:
    """Exact replica of the reference's ternary quantization (fwd value)."""
    try:
        import jax
        import jax.numpy as jnp
        cpu = jax.local_devices(backend="cpu")[0]
        with jax.default_device(cpu):
            Wj = jnp.asarray(W)
            s = jnp.mean(jnp.abs(Wj)) + 1e-8
            q = jnp.round(jnp.clip(Wj / s, -1.0, 1.0)) * s
            return np.asarray(q)
    except Exception:
        s = np.float32(np.mean(np.abs(W.astype(np.float32)))) + np.float32(1e-8)
        return (np.rint(np.clip(W / s, -1.0, 1.0)) * s).astype(np.float32)


def _build_w_eff(W: np.ndarray) -> np.ndarray:
    """[8,P,P] component weights -> effective [C, C] with octonion combine."""
    Wq = _ternary(W)  # (8, P, P)
    return np.einsum("kij,ipq->jpkq", _COMB, Wq).reshape(C, C).astype(np.float32)


def _rope_colperm() -> np.ndarray:
    """colperm[new] = old: within each head, [re0..re63 | im0..im63]."""
    perm = np.zeros(C, dtype=np.int64)
    for h in range(H):
        base = h * HD
        for r in range(HD // 2):
            perm[base + r] = base + 2 * r
            perm[base + HD // 2 + r] = base + 2 * r + 1
    return perm


def prep_inputs(inputs: dict, T: int) -> list[dict]:
    """Build the 8 per-core input maps from the full problem inputs."""
    NT = B * T
    x = np.asarray(inputs["x"], np.float32)[:, :T, :]
    cos = np.asarray(inputs["freqs_cos"], np.float32)[:T]   # [T, 64]
    sin = np.asarray(inputs["freqs_sin"], np.float32)[:T]

    wq_eff = _build_w_eff(np.asarray(inputs["wq"], np.float32))
    wk_eff = _build_w_eff(np.asarray(inputs["wk"], np.float32))
    wv_eff = _build_w_eff(np.asarray(inputs["wv"], np.float32))
    wo_eff = _build_w_eff(np.asarray(inputs["wo"], np.float32))

    perm = _rope_colperm()
    wq_eff = wq_eff[:, perm] * np.float32(HD ** -0.5)
    wk_eff = wk_eff[:, perm]

    # xT [C, NT] packed partition-major -> [128, KT, NT]
    xt = np.ascontiguousarray(
        x.reshape(NT, C).T.reshape(KT, 128, NT).transpose(1, 0, 2).astype(BF16))

    # rope tables, duplicated-half layout [128, T]
    cosd = np.empty((128, T), np.float32)
    cosd[0:64] = cos.T
    cosd[64:128] = cos.T
    sind = np.empty((128, T), np.float32)
    sind[0:64] = -sin.T
    sind[64:128] = sin.T
    cosd = cosd.astype(BF16)
    sind = sind.astype(BF16)

    tri = np.triu(np.ones((128, 128), np.float32)).astype(BF16)  # [s,q] s<=q

    def blocks(w_eff: np.ndarray, c: int) -> np.ndarray:
        blk = w_eff[:, c * P:(c + 1) * P]                  # [C, 256]
        return np.ascontiguousarray(
            blk.reshape(KT, 128, P).transpose(1, 0, 2).astype(BF16))

    in_maps = []
    for c in range(N_CORES):
        # o-proj row-block for component c: [256, C] -> [128, 2, C]
        wo_rows = np.ascontiguousarray(
            wo_eff[c * P:(c + 1) * P, :].reshape(2, 128, C)
            .transpose(1, 0, 2).astype(BF16))
        in_maps.append({
            "xt": xt,
            "wq": blocks(wq_eff, c),
            "wk": blocks(wk_eff, c),
            "wv": blocks(wv_eff, c),
            "wo": wo_rows,
            "cosd": cosd,
            "sind": sind,
            "tri": tri,
        })
    return in_maps


# ------------------------------------------------------------------- weaver
def weave(*streams):
    """Fairly interleave unit streams by cumulative weight.

    Each stream is a list of (weight, fn). Executes every fn exactly once,
    preserving per-stream order, picking at each step the least-progressed
    stream by weight fraction.
    """
    streams = [s for s in streams if s]
    total = [max(sum(w for w, _ in s), 1e-9) for s in streams]
    done = [0.0] * len(streams)
    idx = [0] * len(streams)
    while True:
        cand = [i for i in range(len(streams)) if idx[i] < len(streams[i])]
        if not cand:
            break
        i = min(cand, key=lambda i: done[i] / total[i])
        w, fn = streams[i][idx[i]]
        idx[i] += 1
        done[i] += w
        fn()


# ------------------------------------------------------------- device program
def build_nc(T: int = T_FULL, n_cores: int = N_CORES):
    NT = B * T
    ST = T // 128            # 16 s-tiles per batch
    NST = NT // 128          # 32
    TCH = 512                # token chunk
    NCH = NT // TCH          # 8
    SLAB = 1024              # y-accumulator slab width (2 psum banks)
    bf16 = mybir.dt.bfloat16
    f32 = mybir.dt.float32
    EXP = mybir.ActivationFunctionType.Exp

    nc = bacc.Bacc("TRN2", target_bir_lowering=False, debug=False,
                   num_devices=n_cores)

    xt_d = nc.dram_tensor("xt", [128, KT, NT], bf16, kind="ExternalInput")
    wq_d = nc.dram_tensor("wq", [128, KT, P], bf16, kind="ExternalInput")
    wk_d = nc.dram_tensor("wk", [128, KT, P], bf16, kind="ExternalInput")
    wv_d = nc.dram_tensor("wv", [128, KT, P], bf16, kind="ExternalInput")
    wo_d = nc.dram_tensor("wo", [128, 2, C], bf16, kind="ExternalInput")
    cos_d = nc.dram_tensor("cosd", [128, T], bf16, kind="ExternalInput")
    sin_d = nc.dram_tensor("sind", [128, T], bf16, kind="ExternalInput")
    tri_d = nc.dram_tensor("tri", [128, 128], bf16, kind="ExternalInput")
    out_d = nc.dram_tensor("outt", [C, NT], bf16, kind="ExternalOutput")

    with tile.TileContext(nc) as tc:
        with (
            tc.tile_pool(name="consts", bufs=1) as consts,
            tc.tile_pool(name="persist", bufs=1) as persist,
            tc.tile_pool(name="ptlo", bufs=2) as ptlo,       # pt j=0..3
            tc.tile_pool(name="pthi", bufs=1) as pthi,       # pt j=4..15
            tc.tile_pool(name="ptsum", bufs=2) as ptsum_pool,
            tc.tile_pool(name="recipb", bufs=1) as recipb_pool,
            tc.tile_pool(name="sps", bufs=1, space="PSUM") as sps,
            tc.tile_pool(name="yslab", bufs=1, space="PSUM") as yslab_pool,
            tc.tile_pool(name="denb", bufs=1, space="PSUM") as denb_pool,
        ):
            # ---------------- persistent SBUF state
            wo_s = consts.tile([128, 2, C], bf16, tag="wo")
            tri_s = consts.tile([128, 128], bf16, tag="tri")
            ones_s = consts.tile([128, 128], bf16, tag="ones")
            qt_s = persist.tile([128, 2, NT], bf16, tag="qt")  # [d, head, tok]
            kt_s = persist.tile([128, 2, NT], bf16, tag="kt")
            v_s = persist.tile([128, NST, 2, 128], bf16, tag="v")
            yst = [persist.tile([128, T], bf16, tag=f"yst{p}",
                                name=f"yst{p}") for p in range(4)]

            # mutable emission state
            st = {"pt": {}, "ptsum": {}, "recipb": {}, "yslab": {},
                  "xt": {}, "cs": {}, "osb": {}}

            # ============ emission helpers =================================
            def S_ptsum(p, j):
                """ptsum accumulate (bf16; validated vs reference)."""
                pt = st["pt"][(p, j)]
                if j == 0:
                    pts = ptsum_pool.tile([128, T], bf16, tag="ptsum")
                    st["ptsum"][p] = pts
                    nc.vector.tensor_copy(pts[:, :], pt[:, :])
                else:
                    pts = st["ptsum"][p]
                    nc.vector.tensor_add(pts[:, 128 * j:T],
                                         pts[:, 128 * j:T], pt[:, :])

            def S_group(p, j, cclo, cchi):
                """S^T matmuls for chunks cclo..cchi (one 1024 psum group) of
                tile j + a single exp over the group + diag mask if covered."""
                b, a = p // 2, p % 2
                base = b * T
                kh = kt_s[:, a, base + 128 * j: base + 128 * (j + 1)]
                pt = st["pt"][(p, j)]
                gbase = 1024 * (cclo // 2)
                ps = sps.tile([128, 1024], f32, tag="ps_s")
                qlo = max(512 * cclo, 128 * j)
                qhi = 512 * (cchi + 1)
                for cc in range(cclo, cchi + 1):
                    q0 = max(512 * cc, 128 * j)
                    q1 = 512 * (cc + 1)
                    nc.tensor.matmul(
                        ps[:, q0 - gbase: q1 - gbase], lhsT=kh,
                        rhs=qt_s[:, a, base + q0: base + q1],
                        start=True, stop=True)
                nc.scalar.activation(
                    out=pt[:, qlo - 128 * j: qhi - 128 * j],
                    in_=ps[:, qlo - gbase: qhi - gbase], func=EXP)
                if cclo <= j // 4 <= cchi:
                    # causal mask on the diagonal 128 block
                    nc.vector.tensor_mul(pt[:, 0:128], pt[:, 0:128], tri_s[:])

            def S_groups_for(j, cmax):
                """Chunk ranges j//4..cmax grouped into 1024-col psum tiles."""
                out = []
                c = j // 4
                while c <= cmax:
                    ghi = min(2 * (c // 2) + 1, cmax)
                    out.append((c, ghi))
                    c = ghi + 1
                return out

            def S_unit(p, j, sink, cmax=3):
                """S tile j as 1-2 weave units (one per 1024 psum group; the
                single-buffered sps slot then never stalls the PE within a
                unit). Use cmax<3 when the last q-chunk's qt/kt writers are
                woven into the same stretch; finish via S_tail after them."""
                groups = S_groups_for(j, cmax)

                def u_first(p=p, j=j):
                    pt_pool = ptlo if j < 4 else pthi
                    ptw = T - 128 * j
                    pt = pt_pool.tile([128, ptw], bf16, tag=f"pt{j}")
                    st["pt"][(p, j)] = pt
                    if groups:
                        S_group(p, j, *groups[0])
                    if len(groups) == 1 and cmax == 3:
                        S_ptsum(p, j)
                w0 = (512 * (groups[0][1] + 1) - max(512 * groups[0][0],
                                                     128 * j)) / 2400.0 \
                    if groups else 0.05
                sink.append((max(w0, 0.05), u_first))
                if len(groups) > 1:
                    def u_second(p=p, j=j):
                        S_group(p, j, *groups[1])
                        if cmax == 3:
                            S_ptsum(p, j)
                    w1 = (512 * (groups[1][1] + 1)
                          - max(512 * groups[1][0], 128 * j)) / 2400.0
                    sink.append((max(w1, 0.05), u_second))

            def S_tail(p, cdone):
                """Sequentially finish chunks cdone+1..3 for every tile."""
                for j in range(16):
                    S_group(p, j, cdone + 1, 3)
                    S_ptsum(p, j)

            def PV_unit(p, slab, j, sink):
                """V-stationary PV matmuls for tile j into the y-slab."""
                def u(p=p, slab=slab, j=j):
                    b, a = p // 2, p % 2
                    qlo = slab * SLAB
                    jmax = 8 if slab == 0 else 16
                    if j == 0:
                        ys = yslab_pool.tile([128, SLAB], f32, tag="yslab")
                        st["yslab"][(p, slab)] = ys
                    ys = st["yslab"][(p, slab)]
                    pt = st["pt"][(p, j)]
                    for cc in range(max(2 * slab, j // 4), 2 * slab + 2):
                        q0 = max(512 * cc, 128 * j)
                        q1 = 512 * (cc + 1)
                        nc.tensor.matmul(
                            ys[:, q0 - qlo: q1 - qlo],
                            lhsT=v_s[:, b * ST + j, a, :],
                            rhs=pt[:, q0 - 128 * j: q1 - 128 * j],
                            start=(j == 0),
                            stop=(j == min(jmax - 1, 4 * cc + 3)))
                w = (512 * (2 * slab + 2) - max(512 * 2 * slab, 128 * j)) \
                    / 2400.0
                sink.append((max(w, 0.05), u))

            def denb_unit(p, slab, sink):
                """ones-matmul partition-sum+broadcast of ptsum, then recip."""
                def u(p=p, slab=slab):
                    rb = recipb_pool.tile([128, SLAB], f32, tag="recipb")
                    st["recipb"][(p, slab)] = rb
                    pts = st["ptsum"][p]
                    for cc in range(2 * slab, 2 * slab + 2):
                        dn = denb_pool.tile([128, 512], f32, tag="denb")
                        nc.tensor.matmul(dn[:], lhsT=ones_s[:],
                                         rhs=pts[:, 512 * cc: 512 * (cc + 1)],
                                         start=True, stop=True)
                        # ~5x faster than reciprocal(); denominators are
                        # >= exp(diag)/2 so no denorm/inf edge cases
                        nc.vector.reciprocal_approx_fast(
                            out=rb[:, 512 * cc - slab * SLAB:
                                   512 * (cc + 1) - slab * SLAB], in_=dn[:])
                sink.append((1.0, u))

            def ymul_unit(p, slab, sink):
                """Normalize the finished y-slab into the ystage (DVE)."""
                def u(p=p, slab=slab):
                    ys = st["yslab"][(p, slab)]
                    rb = st["recipb"][(p, slab)]
                    nc.vector.tensor_mul(
                        yst[p][:, slab * SLAB:(slab + 1) * SLAB], ys[:], rb[:])
                # weight inflated so the weaver inserts filler after it,
                # giving the DVE time before the next slab's first matmul
                sink.append((2.0, u))

            def attn_core(p, next_S, next_cmax=3):
                """Safe-ordered core stream for one stretch.

                PV(p, slab1, j) strictly precedes S(p+1, j) for j>=4 so the
                single-buffered pt slot reuse never creates a wait cycle.
                """
                s = []
                if next_S is not None:
                    S_unit(next_S, 0, s, next_cmax)
                    S_unit(next_S, 1, s, next_cmax)
                for j in range(8):
                    PV_unit(p, 0, j, s)
                if next_S is not None:
                    S_unit(next_S, 2, s, next_cmax)
                    S_unit(next_S, 3, s, next_cmax)
                denb_unit(p, 0, s)
                ymul_unit(p, 0, s)
                for j in range(4):
                    PV_unit(p, 1, j, s)
                for j in range(4, 16):
                    PV_unit(p, 1, j, s)
                    if next_S is not None:
                        S_unit(next_S, j, s, next_cmax)
                denb_unit(p, 1, s)
                ymul_unit(p, 1, s)
                return s

            # ================= phase 1: projections =========================
            with (
                tc.tile_pool(name="projw", bufs=1) as projw,
                tc.tile_pool(name="xts", bufs=2) as xts_pool,
                tc.tile_pool(name="cstile", bufs=2) as cs_pool,
                tc.tile_pool(name="rope", bufs=2) as rope_pool,
                tc.tile_pool(name="ps1", bufs=2, space="PSUM") as ps1,
                tc.tile_pool(name="psv", bufs=1, space="PSUM") as psv,
            ):
                wq_s = projw.tile([128, KT, P], bf16, tag="wq")
                wk_s = projw.tile([128, KT, P], bf16, tag="wk")
                wv_s = projw.tile([128, KT, P], bf16, tag="wv")

                def load_xt(ch):
                    xs = xts_pool.tile([128, KT, TCH], bf16, tag="xt")
                    st["xt"][ch] = xs
                    nc.sync.dma_start(
                        out=xs, in_=xt_d.ap()[:, :, ch * TCH:(ch + 1) * TCH])

                def load_cs(ch):
                    pos0 = (ch * TCH) % T
                    cch = cs_pool.tile([128, TCH], bf16, tag="cos")
                    sch = cs_pool.tile([128, TCH], bf16, tag="sin")
                    st["cs"][ch] = (cch, sch)
                    nc.sync.dma_start(out=cch,
                                      in_=cos_d.ap()[:, pos0:pos0 + TCH])
                    nc.sync.dma_start(out=sch,
                                      in_=sin_d.ap()[:, pos0:pos0 + TCH])

                # --- preload DMAs, consumption-ordered
                xt0 = xts_pool.tile([128, KT, TCH], bf16, tag="xt")
                st["xt"][0] = xt0
                for kq in range(0, KT, 4):
                    nc.sync.dma_start(out=wq_s[:, kq:kq + 4, :],
                                      in_=wq_d.ap()[:, kq:kq + 4, :])
                    nc.sync.dma_start(out=xt0[:, kq:kq + 4, :],
                                      in_=xt_d.ap()[:, kq:kq + 4, 0:TCH])
                load_cs(0)
                nc.sync.dma_start(out=wk_s, in_=wk_d.ap())
                load_xt(1)
                nc.sync.dma_start(out=wv_s, in_=wv_d.ap())
                nc.sync.dma_start(out=tri_s, in_=tri_d.ap())
                nc.vector.memset(ones_s[:], 1.0)
                nc.sync.dma_start(out=wo_s, in_=wo_d.ap())

                def proj_units(ch):
                    """8 units interleaved qk/v: each v chain is followed by
                    a q/k chain so the single-buffered psv copy hides."""
                    t0 = ch * TCH
                    qk_units = []
                    for a in range(2):
                        for w_s, dst in ((wq_s, qt_s), (wk_s, kt_s)):
                            def u(w_s=w_s, dst=dst, a=a, t0=t0, ch=ch):
                                xt_sb = st["xt"][ch]
                                cch, sch = st["cs"][ch]
                                ps = ps1.tile([128, TCH], f32, tag="psq")
                                for k in range(KT):
                                    nc.tensor.matmul(
                                        ps[:],
                                        lhsT=w_s[:, k, a * 128:(a + 1) * 128],
                                        rhs=xt_sb[:, k, :],
                                        start=(k == 0), stop=(k == KT - 1))
                                q_sb = rope_pool.tile([128, TCH], bf16,
                                                      tag="qsb")
                                nc.scalar.copy(out=q_sb[:], in_=ps[:])
                                # partition-half swap must go through DMA
                                qsw = rope_pool.tile([128, TCH], bf16,
                                                     tag="qsw")
                                nc.sync.dma_start(out=qsw[0:64, :],
                                                  in_=q_sb[64:128, :])
                                nc.sync.dma_start(out=qsw[64:128, :],
                                                  in_=q_sb[0:64, :])
                                d = dst[:, a, t0:t0 + TCH]
                                nc.vector.tensor_mul(d, q_sb[:], cch[:])
                                t2 = rope_pool.tile([128, TCH], bf16, tag="t2")
                                nc.vector.tensor_mul(t2[:], qsw[:], sch[:])
                                nc.vector.tensor_add(d, d, t2[:])
                            qk_units.append((3.6, u))
                    v_units = []
                    for stt in range(TCH // 128):
                        def u(stt=stt, t0=t0, ch=ch):
                            xt_sb = st["xt"][ch]
                            stg = t0 // 128 + stt
                            ps = psv.tile([128, P], f32, tag="psv")
                            for k in range(KT):
                                nc.tensor.matmul(
                                    ps[:],
                                    lhsT=xt_sb[:, k, stt * 128:(stt + 1) * 128],
                                    rhs=wv_s[:, k, :],
                                    start=(k == 0), stop=(k == KT - 1))
                            nc.vector.tensor_copy(
                                v_s[:, stg, :, :],
                                ps[:].rearrange("p (a d) -> p a d", a=2))
                        v_units.append((1.8, u))
                    units = []
                    for i in range(4):
                        units.append(qk_units[i])
                        units.append(v_units[i])
                    return units

                def S_stream(p, jlo, jhi):
                    s = []
                    for j in range(jlo, jhi):
                        S_unit(p, j, s)
                    return s

                # --- chunks 0..3 (batch 0) straight through
                # NB: the xt prefetch for ch+2 must be EMITTED after chunk
                # ch's units -- the pool slot-rotation dependency only covers
                # accesses emitted so far, so emitting the DMA first lets it
                # overwrite xt[ch] while chunk ch's matmuls still read it.
                for ch in range(4):
                    if ch + 1 < NCH:
                        load_cs(ch + 1)
                    for _, u in proj_units(ch):
                        u()
                    if ch + 2 < NCH:
                        load_xt(ch + 2)

                # --- chunks 4,5 woven with S(0)
                load_cs(5)
                weave(proj_units(4), S_stream(0, 0, 8))
                load_xt(6)
                load_cs(6)
                weave(proj_units(5), S_stream(0, 8, 16))
                load_xt(7)

                # --- stretch A: PV(0) + S(1) + proj ch6
                load_cs(7)
                weave(attn_core(0, 1), proj_units(6))

                # --- stretch B: PV(1) + S(2) + proj ch7.
                # S(2)'s last q-chunk reads qt/kt tokens written by ch7's
                # rope (same stretch), so those chunks are deferred to a
                # sequential tail emitted after the whole weave.
                weave(attn_core(1, 2, next_cmax=2), proj_units(7))
                S_tail(2, 2)

            # ============ phases 2+3: attention tail + o-proj ===============
            with (
                tc.tile_pool(name="osb", bufs=2) as osb_pool,
                tc.tile_pool(name="opj", bufs=3, space="PSUM") as opj,
            ):
                def oproj_units(b, lchs, dve_frac):
                    """o-proj m-tile chains for token chunks lchs of batch b."""
                    units = []
                    for lch in lchs:
                        for m in range(16):
                            def u(b=b, lch=lch, m=m, dve_frac=dve_frac):
                                if m == 0:
                                    osb = osb_pool.tile([128, 16, 512], bf16,
                                                        tag="osb")
                                    st["osb"][(b, lch)] = osb
                                osb = st["osb"][(b, lch)]
                                po = opj.tile([128, 512], f32, tag="po")
                                q0 = lch * 512
                                for a in range(2):
                                    nc.tensor.matmul(
                                        po[:],
                                        lhsT=wo_s[:, a, m * 128:(m + 1) * 128],
                                        rhs=yst[2 * b + a][:, q0:q0 + 512],
                                        start=(a == 0), stop=(a == 1))
                                if (m % 4) < 4 * dve_frac:
                                    nc.vector.tensor_copy(osb[:, m, :], po[:])
                                else:
                                    nc.scalar.copy(out=osb[:, m, :], in_=po[:])
                                # DMA per 8-row half so the final half's
                                # drain overlaps the next chunk's compute
                                if m in (7, 15):
                                    mlo = m - 7
                                    tc0 = b * T + lch * 512
                                    nc.sync.dma_start(
                                        out=out_d.ap()[:, tc0:tc0 + 512]
                                        .rearrange("(m p) t -> p m t", p=128)
                                        [:, mlo:m + 1, :],
                                        in_=osb[:, mlo:m + 1, :])
                            units.append((0.43, u))
                    return units

                # --- stretch C: PV(2) + S(3) + o-proj(b0) chunks 0,1
                weave(attn_core(2, 3), oproj_units(0, (0, 1), 0.5))

                # --- stretch D: PV(3) slab0 + o-proj(b0) chunks 2,3
                sD = []
                for j in range(8):
                    PV_unit(3, 0, j, sD)
                denb_unit(3, 0, sD)
                ymul_unit(3, 0, sD)
                weave(sD, oproj_units(0, (2, 3), 0.5))

                # --- stretch E: PV(3) slab1 + o-proj(b1) chunks 0,1
                # (chunks 2,3 need ymul(3,1), the last core unit, so they
                # must come after the whole core stream -- weaving them in
                # would let the PE block on a DVE op whose own PE deps sit
                # behind the blocked instruction)
                sE = []
                for j in range(16):
                    PV_unit(3, 1, j, sE)
                denb_unit(3, 1, sE)
                ymul_unit(3, 1, sE)
                weave(sE, oproj_units(1, (0, 1), 0.5))
                for _, u in oproj_units(1, (2, 3), 0.5):
                    u()

    nc.compile()
    return nc


# ------------------------------------------------------------------ entrypoint
_NC_CACHE: dict = {}


def _get_nc(T: int):
    if T not in _NC_CACHE:
        _NC_CACHE[T] = build_nc(T)
    return _NC_CACHE[T]


def assemble_output(results: list[dict], T: int = T_FULL) -> np.ndarray:
    # unshard = sum of the 8 tensor-parallel partial projections (bf16 -> f32)
    outT = results[0]["outt"].astype(np.float32)                # [C, NT]
    for r in results[1:]:
        outT += r["outt"].astype(np.float32)
    return np.ascontiguousarray(outT.T).reshape(B, T, C).astype(np.float32)


def kernel(**inputs) -> np.ndarray:
    nc = _get_nc(T_FULL)
    in_maps = prep_inputs(inputs, T_FULL)
    res = run_bass_kernel_spmd(nc, in_maps, list(range(N_CORES)))
    return assemble_output(res.results, T_FULL)
